# revision 15
# baseline (speedup 1.0000x reference)
"""Block-sparse attention Trainium2 kernel (8 NeuronCores, SPMD).

Problem: hidden_states [2, 2048, 2048] fp32; Wq/Wk/Wv [2048, 2048]; Wo
[2048, 2048]. 16 heads x 128 dim, block-banded attention (BLOCK=64,
bandwidth 2 -> each 128-query tile attends a 384-key band with two
64x64 invalid corners).

Sharding: core c = (batch b = c//4) x (head group g = c%4, 4 heads).
Each core computes q/k/v projections for its 4 heads (columns of
Wq/Wk/Wv), banded attention, and a partial output through its rows of
Wo. Host sums the 4 partials per batch. No collectives.

Per-core pipeline (all matmuls bf16, fp32 PSUM accumulate; inputs are
pre-transposed/cast to bf16 host-side during sharding):
  Inputs stream in as few, large, ramped DMAs (hT on sync, wq/wk/wo on
  scalar, wv on gpsimd) so the PE is fed from ~5us on; Q/K projection
  accumulations are split into k(0..7)/k(8..15) half-groups so PSUM
  banks rotate at half-arrival granularity during the DMA ramp.
  Attention computes scores TRANSPOSED (S^T = K_band^T . Q-tile via
  lhsT=KT chunk) so exp(S^T) is directly the P^T operand PV needs --
  no PE transposes of P. Row-sums ride as a ones-column appended to V
  (V-hat [128,516]: 4 heads x [128 v | 1]), so PV yields [AO | rowsum];
  normalize+cast on vector, one PE transpose -> AO^T, fused Wo with
  2-tile lag; output stores spread across sync/scalar/gpsimd.
"""

from contextlib import ExitStack

import numpy as np

import concourse.bass as bass
import concourse.mybir as mybir
import concourse.tile as tile
from concourse import bacc
from concourse.bass_utils import run_bass_kernel_spmd
from concourse.masks import make_identity

S = 2048          # sequence length
HID = 2048        # hidden size
HL = 4            # heads per core
D = 128           # head dim
NKT = HID // 128  # 16 contraction tiles
NQ = S // 128     # 16 query tiles
SCALE = float(D) ** -0.5
NEG = -1e30
BF = mybir.dt.bfloat16
F32 = mybir.dt.float32
VW = 129          # per-head V-hat width: 128 v-cols + ones col

# ramped span layouts (in 128-row k-tiles) for the batched input DMAs.
# Each DGE ring holds only 4 outstanding DMAs -- a 5th dma_start BLOCKS
# the issuing engine -- so scalar (which must run exps later) gets
# exactly 4, and sync/gpsimd absorb any ring-waits while otherwise idle.
HT_SPANS = [(k, k + 1) for k in range(16)]
WQK_SPANS = [(2 * j, 2 * j + 2) for j in range(8)]
WV_SPANS = [(2 * j, 2 * j + 2) for j in range(8)]


class Spanned:
    """k-tile indexed access into a list of multi-k-tile SBUF tiles."""

    def __init__(self, tiles, spans, width):
        self.tiles, self.spans, self.width = tiles, spans, width

    def __call__(self, k):
        for t, (a, b) in zip(self.tiles, self.spans):
            if a <= k < b:
                return t[:, (k - a) * self.width : (k - a + 1) * self.width]
        raise IndexError(k)


def _emit_wo(nc, ps_big, osb_pool, AO_T, WO, out, mt, split=False):
    mts = slice(128 * mt, 128 * (mt + 1))
    store_eng = [nc.sync, nc.scalar, nc.gpsimd, nc.sync]
    for nc_ in range(4):
        ns = slice(512 * nc_, 512 * (nc_ + 1))
        ops_ = ps_big.tile([128, 512], mybir.dt.float32, tag="big", name="wops")
        for dk in range(HL):
            nc.tensor.matmul(
                ops_, lhsT=AO_T[dk][:, mts], rhs=WO(dk)[:, ns],
                start=(dk == 0), stop=(dk == HL - 1),
            )
        osb = osb_pool.tile([128, 512], BF, tag="osb", name="osb")
        if split:
            # last tile: halve the PSUM->SBUF copies and stores and fan
            # them across engines so the final drain chain is short
            nc.vector.tensor_copy(osb[:, 0:256], ops_[:, 0:256])
            nc.scalar.copy(osb[:, 256:512], ops_[:, 256:512])
            store_eng[nc_].dma_start(
                out=out[mts, 512 * nc_ : 512 * nc_ + 256], in_=osb[:, 0:256]
            )
            store_eng[(nc_ + 1) % 3].dma_start(
                out=out[mts, 512 * nc_ + 256 : 512 * (nc_ + 1)], in_=osb[:, 256:512]
            )
        else:
            nc.any.tensor_copy(osb, ops_)
            store_eng[nc_].dma_start(out=out[mts, ns], in_=osb)


def build():
    nc = bacc.Bacc()
    # ht = h^T [hidden, seq]; all inputs pre-transposed/cast to bf16
    # host-side during sharding
    ht = nc.declare_dram_parameter("ht", [HID, S], BF, isOutput=False)
    wq = nc.declare_dram_parameter("wq", [HID, HL * D], BF, isOutput=False)
    wk = nc.declare_dram_parameter("wk", [HID, HL * D], BF, isOutput=False)
    wv = nc.declare_dram_parameter("wv", [HID, HL * D], BF, isOutput=False)
    wo = nc.declare_dram_parameter("wo", [HL * D, HID], BF, isOutput=False)
    out = nc.declare_dram_parameter("out", [S, HID], BF, isOutput=True)

    with ExitStack() as ctx:
        tc = ctx.enter_context(tile.TileContext(nc))
        persist = ctx.enter_context(tc.tile_pool(name="persist", bufs=1))
        qk = ctx.enter_context(tc.tile_pool(name="qk", bufs=2))
        work = ctx.enter_context(tc.tile_pool(name="work", bufs=3))
        stats = ctx.enter_context(tc.tile_pool(name="stats", bufs=8))
        osb_pool = ctx.enter_context(tc.tile_pool(name="osb", bufs=3))
        ps_big = ctx.enter_context(tc.tile_pool(name="ps_big", bufs=4, space="PSUM"))
        ps_sc = ctx.enter_context(tc.tile_pool(name="ps_sc", bufs=2, space="PSUM"))
        ps_ao = ctx.enter_context(tc.tile_pool(name="ps_ao", bufs=1, space="PSUM"))
        ps_at = ctx.enter_context(tc.tile_pool(name="ps_at", bufs=1, space="PSUM"))

        # ---- input loads first: few, large, ramped DMAs so transfers
        # start the moment the engines come up. hT rides the sync HWDGE
        # queue, wq/wk (then wo, late) the scalar queue, wv the gpsimd
        # SWDGE queue -- three queues pull concurrently.
        ht_tiles = [
            persist.tile([128, (b - a) * S], BF, tag=f"ht{a}", name=f"ht{a}")
            for a, b in HT_SPANS
        ]
        wq_tiles = [
            persist.tile([128, (b - a) * 512], BF, tag=f"wq{a}", name=f"wq{a}")
            for a, b in WQK_SPANS
        ]
        wk_tiles = [
            persist.tile([128, (b - a) * 512], BF, tag=f"wk{a}", name=f"wk{a}")
            for a, b in WQK_SPANS
        ]
        wv_tiles = [
            persist.tile([128, (b - a) * 512], BF, tag=f"wv{a}", name=f"wv{a}")
            for a, b in WV_SPANS
        ]
        wo_tile = persist.tile([128, HL * HID], BF, tag="wo", name="wo_t")

        def dram3(t, a, b, w):
            return t[128 * a : 128 * b, :].rearrange("(j p) c -> p j c", p=128)

        def sbuf3(t, w):
            return t.rearrange("p (j c) -> p j c", c=w)

        def ht_dma(eng, k):
            eng.dma_start(out=sbuf3(ht_tiles[k], S), in_=dram3(ht, k, k + 1, S))

        def w_dma(eng, tiles, w, i):
            a, b = WQK_SPANS[i]
            eng.dma_start(out=sbuf3(tiles[i], 512), in_=dram3(w, a, b, 512))

        # lo-critical bytes (hT k0-7, lo weight pairs) stream before any
        # hi bytes, split across all three queues: sync takes hT evens,
        # scalar the lo weight pairs then hT odds, gpsimd wv. wo rides
        # the sync ring last; ring-waits only ever park sync/gpsimd (and
        # scalar briefly, long before its first exp).
        for k in (0, 2, 4, 6):
            ht_dma(nc.sync, k)
        for i in range(4):
            w_dma(nc.scalar, wq_tiles, wq, i)
            w_dma(nc.scalar, wk_tiles, wk, i)
        for i in range(4):
            w_dma(nc.gpsimd, wv_tiles, wv, i)
        for k in (1, 3, 5, 7):
            ht_dma(nc.scalar, k)
        for k in (8, 10, 12, 14):
            ht_dma(nc.sync, k)
        for i in range(4, 8):
            w_dma(nc.scalar, wq_tiles, wq, i)
            w_dma(nc.scalar, wk_tiles, wk, i)
        for i in range(4, 8):
            w_dma(nc.gpsimd, wv_tiles, wv, i)
        for k in (9, 11, 13, 15):
            ht_dma(nc.scalar, k)
        nc.sync.dma_start(out=sbuf3(wo_tile, HID), in_=dram3(wo, 0, 4, HID))

        HT = Spanned(ht_tiles, HT_SPANS, S)
        WQ = Spanned(wq_tiles, WQK_SPANS, 512)
        WK = Spanned(wk_tiles, WQK_SPANS, 512)
        WV = Spanned(wv_tiles, WV_SPANS, 512)

        def WO(dk):
            return wo_tile[:, dk * HID : (dk + 1) * HID]

        # HAM warm-up: dependency-free matmuls at t~3.5us flip the PE
        # clock gate to 2.4GHz before the first DMA-paced projections
        zw = persist.tile([128, 128], BF, tag="zw")
        nc.vector.memset(zw, 0.0)
        warm_ps = ps_ao.tile([128, 128], F32, tag="ao", name="warm_ps")
        for _ in range(40):
            nc.tensor.matmul(warm_ps, lhsT=zw, rhs=zw, start=True, stop=True)

        # transposed additive corner masks, layout [k, (chunk, q)]
        mask_int = persist.tile([128, 384], F32, tag="mask_int")
        nc.vector.memset(mask_int, 0.0)
        nc.vector.memset(mask_int[0:64, 64:128], NEG)
        nc.vector.memset(mask_int[64:128, 256:320], NEG)
        mask_lo = persist.tile([128, 256], F32, tag="mask_lo")
        nc.vector.memset(mask_lo, 0.0)
        nc.vector.memset(mask_lo[64:128, 128:192], NEG)
        mask_hi = persist.tile([128, 256], F32, tag="mask_hi")
        nc.vector.memset(mask_hi, 0.0)
        nc.vector.memset(mask_hi[0:64, 64:128], NEG)

        ident = persist.tile([128, 128], BF, tag="ident")
        make_identity(nc, ident)

        # V-hat tiles [128, 4*129]: per head 128 v-cols + a ones column
        # (the ones column makes PV also produce the softmax row-sums)
        V = [persist.tile([128, HL * VW], BF, tag=f"v{t}", name=f"v{t}") for t in range(NQ)]
        for t in range(NQ):
            nc.gpsimd.memset(
                V[t].rearrange("p (h x) -> p h x", x=VW)[:, :, 128:129], 1.0
            )

        AO_T = [persist.tile([128, S], BF, tag=f"ao{hh}", name=f"ao{hh}") for hh in range(HL)]

        # ---- head-0 + V projections, phased by k-half so the PE
        # stream consumes data in DMA-arrival order during the input
        # ramp (~41us of PE work needs only the k0-7 half of hT).
        # Heads 1-3 project full-depth right before their attention;
        # the scheduler hoists those N=512 matmuls into the previous
        # head's attention stalls. Keeping the dense projection phases
        # short also avoids the sustained-power P0 downclock.
        QTs = [None] * HL
        KTs = [None] * HL
        for hh in range(2):
            QTs[hh] = qk.tile([128, S], BF, tag="q", name=f"qt{hh}", bufs=3)
            KTs[hh] = qk.tile([128, S], BF, tag="k", name=f"kt{hh}", bufs=3)

        def wave(specs, k0, k1):
            # 4 PSUM accumulation groups advance through k in lockstep so
            # the PE FIFO order matches the k-tile DMA arrival order --
            # each arriving k-tile unlocks len(specs) ready matmuls
            pss = [
                ps_big.tile([128, 512], F32, tag="big", name=f"wv{i}")
                for i in range(len(specs))
            ]
            for k in range(k0, k1):
                for (mm, _), ps in zip(specs, pss):
                    mm(k, ps, k == k0, k == k1 - 1)
            for (_, merge), ps in zip(specs, pss):
                merge(ps)

        def q_spec(hh, mc, lo):
            hs_ = slice(128 * hh, 128 * (hh + 1))
            ms = slice(512 * mc, 512 * (mc + 1))

            def mm(k, ps, st, sp):
                nc.tensor.matmul(ps, lhsT=WQ(k)[:, hs_], rhs=HT(k)[:, ms],
                                 start=st, stop=sp)

            def merge(ps):
                # fold the 1/sqrt(d) scaling into Q; the lo half-sum
                # parks in-place in the bf16 destination
                if lo:
                    nc.vector.tensor_scalar_mul(QTs[hh][:, ms], ps, SCALE)
                else:
                    nc.vector.scalar_tensor_tensor(
                        QTs[hh][:, ms], ps, SCALE, QTs[hh][:, ms],
                        op0=mybir.AluOpType.mult, op1=mybir.AluOpType.add,
                    )

            return mm, merge

        def k_spec(hh, mc, lo):
            hs_ = slice(128 * hh, 128 * (hh + 1))
            ms = slice(512 * mc, 512 * (mc + 1))

            def mm(k, ps, st, sp):
                nc.tensor.matmul(ps, lhsT=WK(k)[:, hs_], rhs=HT(k)[:, ms],
                                 start=st, stop=sp)

            def merge(ps):
                if lo:
                    nc.vector.tensor_copy(KTs[hh][:, ms], ps)
                else:
                    nc.vector.tensor_add(KTs[hh][:, ms], ps, KTs[hh][:, ms])

            return mm, merge

        def v_spec(t, lo):
            ts_ = slice(128 * t, 128 * (t + 1))
            vview = V[t].rearrange("p (h x) -> p h x", x=VW)[:, :, 0:128]

            def mm(k, ps, st, sp):
                nc.tensor.matmul(ps, lhsT=HT(k)[:, ts_], rhs=WV(k),
                                 start=st, stop=sp)

            def merge(ps):
                psv = ps.rearrange("p (h x) -> p h x", x=128)
                if lo:
                    nc.vector.tensor_copy(vview, psv)
                else:
                    nc.vector.tensor_add(vview, psv, vview)

            return mm, merge

        # lo phases: everything needing only k0-7 + the lo weight pairs
        for hh in range(2):
            wave([q_spec(hh, mc, True) for mc in range(4)], 0, 8)
            wave([k_spec(hh, mc, True) for mc in range(4)], 0, 8)
        for tb in range(4):
            wave([v_spec(4 * tb + j, True) for j in range(4)], 0, 8)
        # hi phases
        wave([q_spec(0, mc, False) for mc in range(4)], 8, 16)
        wave([k_spec(0, mc, False) for mc in range(4)], 8, 16)
        for tb in range(4):
            wave([v_spec(4 * tb + j, False) for j in range(4)], 8, 16)

        def emit_qk_hi(hh):
            wave([q_spec(hh, mc, False) for mc in range(4)], 8, 16)
            wave([k_spec(hh, mc, False) for mc in range(4)], 8, 16)

        # ---- per-head: full-depth QK projection (heads 1-3), then
        # attention; Wo fused into the last head's loop
        for hh in range(HL):
            if hh == 1:
                # head 1's hi halves hoist into head 0's attention stalls
                emit_qk_hi(1)
            elif hh > 1:
                hs_ = slice(128 * hh, 128 * (hh + 1))
                QTs[hh] = qk.tile([128, S], BF, tag="q", name=f"qt{hh}", bufs=3)
                KTs[hh] = qk.tile([128, S], BF, tag="k", name=f"kt{hh}", bufs=3)
                for mc in range(4):
                    ms = slice(512 * mc, 512 * (mc + 1))
                    qps = ps_big.tile([128, 512], F32, tag="big", name="qps")
                    for k in range(NKT):
                        nc.tensor.matmul(
                            qps, lhsT=WQ(k)[:, hs_], rhs=HT(k)[:, ms],
                            start=(k == 0), stop=(k == NKT - 1),
                        )
                    nc.vector.tensor_scalar_mul(QTs[hh][:, ms], qps, SCALE)
                    kps = ps_big.tile([128, 512], F32, tag="big", name="kps")
                    for k in range(NKT):
                        nc.tensor.matmul(
                            kps, lhsT=WK(k)[:, hs_], rhs=HT(k)[:, ms],
                            start=(k == 0), stop=(k == NKT - 1),
                        )
                    nc.vector.tensor_copy(KTs[hh][:, ms], kps)
            QT, KT = QTs[hh], KTs[hh]
            for qt in range(NQ):
                kt0 = max(0, qt - 1)
                kt1 = min(NQ, qt + 2)
                nch = kt1 - kt0
                qs = slice(128 * qt, 128 * (qt + 1))
                # transposed scores: one PSUM group, chunk c in cols
                # [128c, 128c+128) = K-tile (kt0+c) against this q-tile
                scps = ps_sc.tile([128, nch * 128], F32, tag="sc", name="scps")
                for c in range(nch):
                    kts = slice(128 * (kt0 + c), 128 * (kt0 + c + 1))
                    nc.tensor.matmul(
                        scps[:, 128 * c : 128 * (c + 1)],
                        lhsT=KT[:, kts], rhs=QT[:, qs],
                        start=(c == 0), stop=(c == nch - 1),
                    )
                mask = mask_lo if qt == 0 else (mask_hi if qt == NQ - 1 else mask_int)
                sc = work.tile([128, nch * 128], F32, tag="scsb", name="sc")
                # copy PSUM->SBUF fused with the corner mask add
                nc.vector.tensor_add(sc, scps, mask)
                # scores are O(+-8) so exp needs no max subtraction
                # (softmax is shift-invariant; fp32 exp is safe here)
                pt = work.tile([128, nch * 128], BF, tag="pt", name="pt")
                nc.scalar.activation(pt, sc, mybir.ActivationFunctionType.Exp)
                # PV: P^T chunks are the stationary operand; the ones
                # column of V-hat accumulates the row-sums in col 128
                aops = ps_ao.tile([128, VW], F32, tag="ao", name="aops")
                for c in range(nch):
                    nc.tensor.matmul(
                        aops,
                        lhsT=pt[:, 128 * c : 128 * (c + 1)],
                        rhs=V[kt0 + c][:, VW * hh : VW * hh + VW],
                        start=(c == 0), stop=(c == nch - 1),
                    )
                rcp = stats.tile([128, 1], F32, tag="rcp")
                nc.vector.reciprocal(rcp, aops[:, 128:129])
                aosb = work.tile([128, 128], BF, tag="aosb", name="aosb")
                nc.vector.tensor_scalar_mul(aosb, aops[:, 0:128], rcp)
                atps = ps_at.tile([128, 128], BF, tag="at", name="atps")
                nc.tensor.transpose(atps, aosb, ident)
                nc.scalar.copy(AO_T[hh][:, qs], atps)

                # fuse the output projection into the last head's loop
                # with a 1-tile lag so Wo matmuls are never gated on the
                # in-flight softmax chain of the same tile
                if hh == HL - 1 and qt >= 1:
                    _emit_wo(nc, ps_big, osb_pool, AO_T, WO, out, qt - 1)
        _emit_wo(nc, ps_big, osb_pool, AO_T, WO, out, NQ - 1, split=True)

    if not nc.is_finalized():
        nc.finalize()
    return nc


_NC = None


def _get_nc():
    global _NC
    if _NC is None:
        _NC = build()
    return _NC


def _in_maps(hidden_states, Wq, Wk, Wv, Wo):
    import ml_dtypes

    bf = ml_dtypes.bfloat16
    hs = np.asarray(hidden_states, dtype=np.float32)
    Wq = np.asarray(Wq, dtype=np.float32)
    Wk = np.asarray(Wk, dtype=np.float32)
    Wv = np.asarray(Wv, dtype=np.float32)
    Wo = np.asarray(Wo, dtype=np.float32)
    maps = []
    for c in range(8):
        b, g = divmod(c, 4)
        sl = slice(512 * g, 512 * (g + 1))
        maps.append(
            {
                "ht": np.ascontiguousarray(hs[b].T).astype(bf),
                "wq": np.ascontiguousarray(Wq[:, sl]).astype(bf),
                "wk": np.ascontiguousarray(Wk[:, sl]).astype(bf),
                "wv": np.ascontiguousarray(Wv[:, sl]).astype(bf),
                "wo": np.ascontiguousarray(Wo[sl, :]).astype(bf),
            }
        )
    return maps


def _gather(results):
    outs = [np.asarray(results[c]["out"]).astype(np.float32) for c in range(8)]
    return np.stack(
        [outs[0] + outs[1] + outs[2] + outs[3],
         outs[4] + outs[5] + outs[6] + outs[7]]
    )


def run(in_maps, trace=False, **kw):
    nc = _get_nc()
    return run_bass_kernel_spmd(nc, in_maps, core_ids=list(range(8)), trace=trace, **kw)


def kernel(hidden_states, Wq, Wk, Wv, Wo):
    maps = _in_maps(hidden_states, Wq, Wk, Wv, Wo)
    res = run(maps)
    return _gather(res.results)


# revision 16
# speedup vs baseline: 1.0099x; 1.0099x over previous
"""Block-sparse attention Trainium2 kernel (8 NeuronCores, SPMD).

Problem: hidden_states [2, 2048, 2048] fp32; Wq/Wk/Wv [2048, 2048]; Wo
[2048, 2048]. 16 heads x 128 dim, block-banded attention (BLOCK=64,
bandwidth 2 -> each 128-query tile attends a 384-key band with two
64x64 invalid corners).

Sharding: core c = (batch b = c//4) x (head group g = c%4, 4 heads).
Each core computes q/k/v projections for its 4 heads (columns of
Wq/Wk/Wv), banded attention, and a partial output through its rows of
Wo. Host sums the 4 partials per batch. No collectives.

Per-core pipeline (all matmuls bf16, fp32 PSUM accumulate; inputs are
pre-transposed/cast to bf16 host-side during sharding):
  Inputs stream in as few, large, ramped DMAs (hT on sync, wq/wk/wo on
  scalar, wv on gpsimd) so the PE is fed from ~5us on; Q/K projection
  accumulations are split into k(0..7)/k(8..15) half-groups so PSUM
  banks rotate at half-arrival granularity during the DMA ramp.
  Attention computes scores TRANSPOSED (S^T = K_band^T . Q-tile via
  lhsT=KT chunk) so exp(S^T) is directly the P^T operand PV needs --
  no PE transposes of P. Row-sums ride as a ones-column appended to V
  (V-hat [128,516]: 4 heads x [128 v | 1]), so PV yields [AO | rowsum];
  normalize+cast on vector, one PE transpose -> AO^T, fused Wo with
  2-tile lag; output stores spread across sync/scalar/gpsimd.
"""

from contextlib import ExitStack

import numpy as np

import concourse.bass as bass
import concourse.mybir as mybir
import concourse.tile as tile
from concourse import bacc
from concourse.bass_utils import run_bass_kernel_spmd
from concourse.masks import make_identity

S = 2048          # sequence length
HID = 2048        # hidden size
HL = 4            # heads per core
D = 128           # head dim
NKT = HID // 128  # 16 contraction tiles
NQ = S // 128     # 16 query tiles
SCALE = float(D) ** -0.5
NEG = -1e30
BF = mybir.dt.bfloat16
F32 = mybir.dt.float32
VW = 129          # per-head V-hat width: 128 v-cols + ones col

# ramped span layouts (in 128-row k-tiles) for the batched input DMAs.
# Each DGE ring holds only 4 outstanding DMAs -- a 5th dma_start BLOCKS
# the issuing engine -- so scalar (which must run exps later) gets
# exactly 4, and sync/gpsimd absorb any ring-waits while otherwise idle.
HT_SPANS = [(k, k + 1) for k in range(16)]
WQK_SPANS = [(2 * j, 2 * j + 2) for j in range(8)]
WV_SPANS = [(2 * j, 2 * j + 2) for j in range(8)]


class Spanned:
    """k-tile indexed access into a list of multi-k-tile SBUF tiles."""

    def __init__(self, tiles, spans, width):
        self.tiles, self.spans, self.width = tiles, spans, width

    def __call__(self, k):
        for t, (a, b) in zip(self.tiles, self.spans):
            if a <= k < b:
                return t[:, (k - a) * self.width : (k - a + 1) * self.width]
        raise IndexError(k)


def _emit_wo(nc, ps_big, osb_pool, AO_T, WO, out, mt, split=False):
    mts = slice(128 * mt, 128 * (mt + 1))
    store_eng = [nc.sync, nc.scalar, nc.gpsimd, nc.sync]
    for nc_ in range(4):
        ns = slice(512 * nc_, 512 * (nc_ + 1))
        ops_ = ps_big.tile([128, 512], mybir.dt.float32, tag="big", name="wops")
        for dk in range(HL):
            nc.tensor.matmul(
                ops_, lhsT=AO_T[dk][:, mts], rhs=WO(dk)[:, ns],
                start=(dk == 0), stop=(dk == HL - 1),
            )
        osb = osb_pool.tile([128, 512], BF, tag="osb", name="osb")
        if split:
            # last tile: halve the PSUM->SBUF copies and stores and fan
            # them across engines so the final drain chain is short
            nc.vector.tensor_copy(osb[:, 0:256], ops_[:, 0:256])
            nc.scalar.copy(osb[:, 256:512], ops_[:, 256:512])
            store_eng[nc_].dma_start(
                out=out[mts, 512 * nc_ : 512 * nc_ + 256], in_=osb[:, 0:256]
            )
            store_eng[(nc_ + 1) % 3].dma_start(
                out=out[mts, 512 * nc_ + 256 : 512 * (nc_ + 1)], in_=osb[:, 256:512]
            )
        else:
            nc.any.tensor_copy(osb, ops_)
            store_eng[nc_].dma_start(out=out[mts, ns], in_=osb)


def build():
    nc = bacc.Bacc()
    # ht = h^T [hidden, seq]; all inputs pre-transposed/cast to bf16
    # host-side during sharding
    ht = nc.declare_dram_parameter("ht", [HID, S], BF, isOutput=False)
    wq = nc.declare_dram_parameter("wq", [HID, HL * D], BF, isOutput=False)
    wk = nc.declare_dram_parameter("wk", [HID, HL * D], BF, isOutput=False)
    wv = nc.declare_dram_parameter("wv", [HID, HL * D], BF, isOutput=False)
    wo = nc.declare_dram_parameter("wo", [HL * D, HID], BF, isOutput=False)
    out = nc.declare_dram_parameter("out", [S, HID], BF, isOutput=True)

    with ExitStack() as ctx:
        tc = ctx.enter_context(tile.TileContext(nc))
        persist = ctx.enter_context(tc.tile_pool(name="persist", bufs=1))
        qk = ctx.enter_context(tc.tile_pool(name="qk", bufs=2))
        work = ctx.enter_context(tc.tile_pool(name="work", bufs=3))
        stats = ctx.enter_context(tc.tile_pool(name="stats", bufs=8))
        osb_pool = ctx.enter_context(tc.tile_pool(name="osb", bufs=3))
        ps_big = ctx.enter_context(tc.tile_pool(name="ps_big", bufs=4, space="PSUM"))
        ps_sc = ctx.enter_context(tc.tile_pool(name="ps_sc", bufs=2, space="PSUM"))
        ps_ao = ctx.enter_context(tc.tile_pool(name="ps_ao", bufs=1, space="PSUM"))
        ps_at = ctx.enter_context(tc.tile_pool(name="ps_at", bufs=1, space="PSUM"))

        # ---- input loads first: few, large, ramped DMAs so transfers
        # start the moment the engines come up. hT rides the sync HWDGE
        # queue, wq/wk (then wo, late) the scalar queue, wv the gpsimd
        # SWDGE queue -- three queues pull concurrently.
        ht_tiles = [
            persist.tile([128, (b - a) * S], BF, tag=f"ht{a}", name=f"ht{a}")
            for a, b in HT_SPANS
        ]
        wq_tiles = [
            persist.tile([128, (b - a) * 512], BF, tag=f"wq{a}", name=f"wq{a}")
            for a, b in WQK_SPANS
        ]
        wk_tiles = [
            persist.tile([128, (b - a) * 512], BF, tag=f"wk{a}", name=f"wk{a}")
            for a, b in WQK_SPANS
        ]
        wv_tiles = [
            persist.tile([128, (b - a) * 512], BF, tag=f"wv{a}", name=f"wv{a}")
            for a, b in WV_SPANS
        ]
        wo_tile = persist.tile([128, HL * HID], BF, tag="wo", name="wo_t")

        def dram3(t, a, b, w):
            return t[128 * a : 128 * b, :].rearrange("(j p) c -> p j c", p=128)

        def sbuf3(t, w):
            return t.rearrange("p (j c) -> p j c", c=w)

        # hT stays on the sync queue in strict k order (the FIFO PE
        # stream consumes k-serially; queue-splitting reorders arrivals
        # and stalls it). Lo-half weight pairs stream before hi pairs.
        for k in range(NKT):
            nc.sync.dma_start(out=sbuf3(ht_tiles[k], S), in_=dram3(ht, k, k + 1, S))
        for half in range(2):
            for i in range(len(WQK_SPANS)):
                a, b = WQK_SPANS[i]
                if (a < 8) != (half == 0):
                    continue
                nc.scalar.dma_start(out=sbuf3(wq_tiles[i], 512), in_=dram3(wq, a, b, 512))
                nc.scalar.dma_start(out=sbuf3(wk_tiles[i], 512), in_=dram3(wk, a, b, 512))
        for half in range(2):
            for t, (a, b) in zip(wv_tiles, WV_SPANS):
                if (a < 8) != (half == 0):
                    continue
                nc.gpsimd.dma_start(out=sbuf3(t, 512), in_=dram3(wv, a, b, 512))
        nc.sync.dma_start(out=sbuf3(wo_tile, HID), in_=dram3(wo, 0, 4, HID))

        HT = Spanned(ht_tiles, HT_SPANS, S)
        WQ = Spanned(wq_tiles, WQK_SPANS, 512)
        WK = Spanned(wk_tiles, WQK_SPANS, 512)
        WV = Spanned(wv_tiles, WV_SPANS, 512)

        def WO(dk):
            return wo_tile[:, dk * HID : (dk + 1) * HID]

        # HAM warm-up: dependency-free matmuls at t~3.5us flip the PE
        # clock gate to 2.4GHz before the first DMA-paced projections
        zw = persist.tile([128, 128], BF, tag="zw")
        nc.vector.memset(zw, 0.0)
        warm_ps = ps_ao.tile([128, 128], F32, tag="ao", name="warm_ps")
        for _ in range(40):
            nc.tensor.matmul(warm_ps, lhsT=zw, rhs=zw, start=True, stop=True)

        # transposed additive corner masks, layout [k, (chunk, q)]
        mask_int = persist.tile([128, 384], F32, tag="mask_int")
        nc.vector.memset(mask_int, 0.0)
        nc.vector.memset(mask_int[0:64, 64:128], NEG)
        nc.vector.memset(mask_int[64:128, 256:320], NEG)
        mask_lo = persist.tile([128, 256], F32, tag="mask_lo")
        nc.vector.memset(mask_lo, 0.0)
        nc.vector.memset(mask_lo[64:128, 128:192], NEG)
        mask_hi = persist.tile([128, 256], F32, tag="mask_hi")
        nc.vector.memset(mask_hi, 0.0)
        nc.vector.memset(mask_hi[0:64, 64:128], NEG)

        ident = persist.tile([128, 128], BF, tag="ident")
        make_identity(nc, ident)

        # V-hat tiles [128, 4*129]: per head 128 v-cols + a ones column
        # (the ones column makes PV also produce the softmax row-sums)
        V = [persist.tile([128, HL * VW], BF, tag=f"v{t}", name=f"v{t}") for t in range(NQ)]
        for t in range(NQ):
            nc.gpsimd.memset(
                V[t].rearrange("p (h x) -> p h x", x=VW)[:, :, 128:129], 1.0
            )

        AO_T = [persist.tile([128, S], BF, tag=f"ao{hh}", name=f"ao{hh}") for hh in range(HL)]

        # ---- head-0 + V projections, phased by k-half so the PE
        # stream consumes data in DMA-arrival order during the input
        # ramp (~41us of PE work needs only the k0-7 half of hT).
        # Heads 1-3 project full-depth right before their attention;
        # the scheduler hoists those N=512 matmuls into the previous
        # head's attention stalls. Keeping the dense projection phases
        # short also avoids the sustained-power P0 downclock.
        QTs = [None] * HL
        KTs = [None] * HL
        for hh in range(2):
            QTs[hh] = qk.tile([128, S], BF, tag="q", name=f"qt{hh}", bufs=3)
            KTs[hh] = qk.tile([128, S], BF, tag="k", name=f"kt{hh}", bufs=3)

        def wave(specs, k0, k1):
            # 4 PSUM accumulation groups advance through k in lockstep so
            # the PE FIFO order matches the k-tile DMA arrival order --
            # each arriving k-tile unlocks len(specs) ready matmuls
            pss = [
                ps_big.tile([128, 512], F32, tag="big", name=f"wv{i}")
                for i in range(len(specs))
            ]
            for k in range(k0, k1):
                for (mm, _), ps in zip(specs, pss):
                    mm(k, ps, k == k0, k == k1 - 1)
            for (_, merge), ps in zip(specs, pss):
                merge(ps)

        def q_spec(hh, mc, lo):
            hs_ = slice(128 * hh, 128 * (hh + 1))
            ms = slice(512 * mc, 512 * (mc + 1))

            def mm(k, ps, st, sp):
                nc.tensor.matmul(ps, lhsT=WQ(k)[:, hs_], rhs=HT(k)[:, ms],
                                 start=st, stop=sp)

            def merge(ps):
                # fold the 1/sqrt(d) scaling into Q; the lo half-sum
                # parks in-place in the bf16 destination
                if lo:
                    nc.vector.tensor_scalar_mul(QTs[hh][:, ms], ps, SCALE)
                else:
                    nc.vector.scalar_tensor_tensor(
                        QTs[hh][:, ms], ps, SCALE, QTs[hh][:, ms],
                        op0=mybir.AluOpType.mult, op1=mybir.AluOpType.add,
                    )

            return mm, merge

        def k_spec(hh, mc, lo):
            hs_ = slice(128 * hh, 128 * (hh + 1))
            ms = slice(512 * mc, 512 * (mc + 1))

            def mm(k, ps, st, sp):
                nc.tensor.matmul(ps, lhsT=WK(k)[:, hs_], rhs=HT(k)[:, ms],
                                 start=st, stop=sp)

            def merge(ps):
                if lo:
                    nc.vector.tensor_copy(KTs[hh][:, ms], ps)
                else:
                    nc.vector.tensor_add(KTs[hh][:, ms], ps, KTs[hh][:, ms])

            return mm, merge

        def v_spec(t, lo):
            ts_ = slice(128 * t, 128 * (t + 1))
            vview = V[t].rearrange("p (h x) -> p h x", x=VW)[:, :, 0:128]

            def mm(k, ps, st, sp):
                nc.tensor.matmul(ps, lhsT=HT(k)[:, ts_], rhs=WV(k),
                                 start=st, stop=sp)

            def merge(ps):
                psv = ps.rearrange("p (h x) -> p h x", x=128)
                if lo:
                    nc.vector.tensor_copy(vview, psv)
                else:
                    nc.vector.tensor_add(vview, psv, vview)

            return mm, merge

        # lo phases: everything needing only k0-7 + the lo weight pairs
        for hh in range(2):
            wave([q_spec(hh, mc, True) for mc in range(4)], 0, 8)
            wave([k_spec(hh, mc, True) for mc in range(4)], 0, 8)
        for tb in range(4):
            wave([v_spec(4 * tb + j, True) for j in range(4)], 0, 8)
        # hi phases
        wave([q_spec(0, mc, False) for mc in range(4)], 8, 16)
        wave([k_spec(0, mc, False) for mc in range(4)], 8, 16)
        for tb in range(4):
            wave([v_spec(4 * tb + j, False) for j in range(4)], 8, 16)

        def emit_qk_hi(hh):
            wave([q_spec(hh, mc, False) for mc in range(4)], 8, 16)
            wave([k_spec(hh, mc, False) for mc in range(4)], 8, 16)

        # ---- per-head: full-depth QK projection (heads 1-3), then
        # attention; Wo fused into the last head's loop
        for hh in range(HL):
            if hh == 1:
                # head 1's hi halves hoist into head 0's attention stalls
                emit_qk_hi(1)
            elif hh > 1:
                hs_ = slice(128 * hh, 128 * (hh + 1))
                QTs[hh] = qk.tile([128, S], BF, tag="q", name=f"qt{hh}", bufs=3)
                KTs[hh] = qk.tile([128, S], BF, tag="k", name=f"kt{hh}", bufs=3)
                for mc in range(4):
                    ms = slice(512 * mc, 512 * (mc + 1))
                    qps = ps_big.tile([128, 512], F32, tag="big", name="qps")
                    for k in range(NKT):
                        nc.tensor.matmul(
                            qps, lhsT=WQ(k)[:, hs_], rhs=HT(k)[:, ms],
                            start=(k == 0), stop=(k == NKT - 1),
                        )
                    nc.vector.tensor_scalar_mul(QTs[hh][:, ms], qps, SCALE)
                    kps = ps_big.tile([128, 512], F32, tag="big", name="kps")
                    for k in range(NKT):
                        nc.tensor.matmul(
                            kps, lhsT=WK(k)[:, hs_], rhs=HT(k)[:, ms],
                            start=(k == 0), stop=(k == NKT - 1),
                        )
                    nc.vector.tensor_copy(KTs[hh][:, ms], kps)
            QT, KT = QTs[hh], KTs[hh]
            for qt in range(NQ):
                kt0 = max(0, qt - 1)
                kt1 = min(NQ, qt + 2)
                nch = kt1 - kt0
                qs = slice(128 * qt, 128 * (qt + 1))
                # transposed scores: one PSUM group, chunk c in cols
                # [128c, 128c+128) = K-tile (kt0+c) against this q-tile
                scps = ps_sc.tile([128, nch * 128], F32, tag="sc", name="scps")
                for c in range(nch):
                    kts = slice(128 * (kt0 + c), 128 * (kt0 + c + 1))
                    nc.tensor.matmul(
                        scps[:, 128 * c : 128 * (c + 1)],
                        lhsT=KT[:, kts], rhs=QT[:, qs],
                        start=(c == 0), stop=(c == nch - 1),
                    )
                mask = mask_lo if qt == 0 else (mask_hi if qt == NQ - 1 else mask_int)
                sc = work.tile([128, nch * 128], F32, tag="scsb", name="sc")
                # copy PSUM->SBUF fused with the corner mask add
                nc.vector.tensor_add(sc, scps, mask)
                # scores are O(+-8) so exp needs no max subtraction
                # (softmax is shift-invariant; fp32 exp is safe here)
                pt = work.tile([128, nch * 128], BF, tag="pt", name="pt")
                nc.scalar.activation(pt, sc, mybir.ActivationFunctionType.Exp)
                # PV: P^T chunks are the stationary operand; the ones
                # column of V-hat accumulates the row-sums in col 128
                aops = ps_ao.tile([128, VW], F32, tag="ao", name="aops")
                for c in range(nch):
                    nc.tensor.matmul(
                        aops,
                        lhsT=pt[:, 128 * c : 128 * (c + 1)],
                        rhs=V[kt0 + c][:, VW * hh : VW * hh + VW],
                        start=(c == 0), stop=(c == nch - 1),
                    )
                rcp = stats.tile([128, 1], F32, tag="rcp")
                nc.vector.reciprocal(rcp, aops[:, 128:129])
                aosb = work.tile([128, 128], BF, tag="aosb", name="aosb")
                nc.vector.tensor_scalar_mul(aosb, aops[:, 0:128], rcp)
                atps = ps_at.tile([128, 128], BF, tag="at", name="atps")
                nc.tensor.transpose(atps, aosb, ident)
                nc.scalar.copy(AO_T[hh][:, qs], atps)

                # fuse the output projection into the last head's loop
                # with a 1-tile lag so Wo matmuls are never gated on the
                # in-flight softmax chain of the same tile
                if hh == HL - 1 and qt >= 1:
                    _emit_wo(nc, ps_big, osb_pool, AO_T, WO, out, qt - 1)
        _emit_wo(nc, ps_big, osb_pool, AO_T, WO, out, NQ - 1, split=True)

    if not nc.is_finalized():
        nc.finalize()
    return nc


_NC = None


def _get_nc():
    global _NC
    if _NC is None:
        _NC = build()
    return _NC


def _in_maps(hidden_states, Wq, Wk, Wv, Wo):
    import ml_dtypes

    bf = ml_dtypes.bfloat16
    hs = np.asarray(hidden_states, dtype=np.float32)
    Wq = np.asarray(Wq, dtype=np.float32)
    Wk = np.asarray(Wk, dtype=np.float32)
    Wv = np.asarray(Wv, dtype=np.float32)
    Wo = np.asarray(Wo, dtype=np.float32)
    maps = []
    for c in range(8):
        b, g = divmod(c, 4)
        sl = slice(512 * g, 512 * (g + 1))
        maps.append(
            {
                "ht": np.ascontiguousarray(hs[b].T).astype(bf),
                "wq": np.ascontiguousarray(Wq[:, sl]).astype(bf),
                "wk": np.ascontiguousarray(Wk[:, sl]).astype(bf),
                "wv": np.ascontiguousarray(Wv[:, sl]).astype(bf),
                "wo": np.ascontiguousarray(Wo[sl, :]).astype(bf),
            }
        )
    return maps


def _gather(results):
    outs = [np.asarray(results[c]["out"]).astype(np.float32) for c in range(8)]
    return np.stack(
        [outs[0] + outs[1] + outs[2] + outs[3],
         outs[4] + outs[5] + outs[6] + outs[7]]
    )


def run(in_maps, trace=False, **kw):
    nc = _get_nc()
    return run_bass_kernel_spmd(nc, in_maps, core_ids=list(range(8)), trace=trace, **kw)


def kernel(hidden_states, Wq, Wk, Wv, Wo):
    maps = _in_maps(hidden_states, Wq, Wk, Wv, Wo)
    res = run(maps)
    return _gather(res.results)
